# revision 39
# baseline (speedup 1.0000x reference)
"""NURBS surface evaluation on 8 TRN2 NeuronCores.

Reformulation: the reference einsum
    out[x, y, d] = sum_{l,r} bx[l,x] * cp[ix(l,x), iy(r,y), d] * by[r,y]
is out[:, :, d] = A @ cp[:, :, d] @ B.T once the 4 basis weights per eval
point are scattered into dense (1024, 32) basis matrices A / B.  The left
product is reassociated host-side: T3[d, j, x] = sum_i cp[i, j, d] * A[x, i]
is tiny (3*32*1024), so each core receives its x-slice of T3 pre-computed
and the device does only the million-point right product
    out[x, 3y+d] = sum_j T3[d, j, x] * B[y, j]
as 3 channel matmuls (K=32) per y-chunk.  All device math is fp16 in / f32
PSUM out / fp16 store (rel err ~5e-4 vs the 2e-2 gate); fp16 keeps the PE
single-pass at any moving size and halves DMA traffic.  Sharding: eval-grid
x axis split across 8 cores (128 rows each); output gathered and converted
to f32 on host.
"""

import numpy as np

DEGREE = 3
NCTRL = 32
EOUT = 1024
DIM = 3
EPS = 1e-5
NCORES = 8
ROWS = EOUT // NCORES          # 128 eval rows per core
OUTW = EOUT * DIM              # 3072 interleaved output columns
INW = DIM * ROWS + EOUT        # [T3 | bt] packed input width (1408)

# y-chunks: per chunk the three channel matmuls accumulate into one PSUM
# bank ([128, 3w] f32, so w <= 170) and ONE engine copy moves+interleaves
# all three channels to SBUF (single PSUM-access penalty).  Copies
# alternate DVE/ACT (Pool cannot touch PSUM).  Output DMAs cover groups of
# consecutive chunks; the first group small so its DMA starts early, the
# last small so the final transfer (critical-path tail) is short.
YCH = [(0, 100), (100, 230), (230, 400), (400, 570), (570, 740), (740, 1024)]
DMAG = [[0, 1], [2, 3], [4, 5]]
# per-chunk copy mode: 0 fused-on-DVE, 1 fused-on-ACT, 2 DVE(d0,d1)+ACT(d2),
# 3 ACT(d0,d1)+DVE(d2), 4 DVE(d0)+ACT(d1,d2), 5 ACT(d0)+DVE(d1,d2)
CENG = [0, 1, 0, 1, 0, 3]
PSBUFS = 8
DMAENG = [0, 1, 0]            # out-DMA sequencer per group: 0 = SP, 1 = ACT
INSPLIT = None                # optional input-DMA split column (t3b+early bt)


# ----------------------------------------------------------------- host math
def _normalize_knots(kv):
    kv = np.cumsum(np.where(kv < 0.0, np.float32(1e-4), kv), axis=1,
                   dtype=np.float32)
    return (kv - kv[:, :1]) / (kv[:, -1:] - kv[:, :1])


def _find_spans(ev, kv):
    internal = kv[:, DEGREE:-DEGREE]                      # (S, Ki)
    diff = ev[None, None, :] - internal[:, :, None]       # (S, Ki, E)
    diff = np.where(diff > 1e-8, diff, np.float32(1.0))
    return np.argmin(diff, axis=1) + DEGREE               # (S, E)


def _basis(ev, kv, spans):
    # Cox-de-Boor recursion, mirrors the reference op-for-op in f32.
    S, E = spans.shape
    basis = [np.zeros((S, E), kv.dtype) for _ in range(DEGREE + 1)]
    basis[0] = np.ones((S, E), kv.dtype)
    for k in range(1, DEGREE + 1):
        saved = np.zeros((S, E), kv.dtype)
        for r in range(k):
            left = np.take_along_axis(kv, spans + r + 1, axis=1)
            right = np.take_along_axis(kv, spans + 1 - k + r, axis=1)
            denom = (left - ev) + (ev - right)
            safe = np.where(denom == 0.0, np.float32(1.0), denom)
            temp = np.where(denom == 0.0, np.float32(1e-4), basis[r] / safe)
            basis[r] = saved + (left - ev) * temp
            saved = (ev - right) * temp
        basis[k] = saved
    return np.stack(basis, axis=1)                        # (S, DEGREE+1, E)


def _dense_basis_matrix(knots):
    """(EOUT, NCTRL) dense basis matrix M with M[e, i] the weight of control
    index i at eval point e, replicating the reference gather indices
    (span - 3 - l, wrapped once for negatives)."""
    ev = np.linspace(EPS, 1.0 - EPS, EOUT, dtype=np.float32)
    kv = _normalize_knots(np.asarray(knots, dtype=np.float32))
    spans = _find_spans(ev, kv)
    b = _basis(ev, kv, spans)[0]                          # (DEGREE+1, E)
    sp = spans[0]
    M = np.zeros((EOUT, NCTRL), dtype=np.float32)
    for l in range(DEGREE + 1):
        idx = sp - (DEGREE + l)
        idx = np.where(idx < 0, idx + NCTRL, idx)
        M[np.arange(EOUT), idx] = b[l]
    return M


# ------------------------------------------------------------- device kernel
_NC_CACHE = {}


def _build_nc():
    if "nc" in _NC_CACHE:
        return _NC_CACHE["nc"]
    from contextlib import ExitStack
    import concourse.bacc as bacc
    import concourse.tile as tile
    import concourse.mybir as mybir

    f32 = mybir.dt.float32
    f16 = mybir.dt.float16
    # Bacc (not plain Bass): its finalize() runs generate_event_semaphores,
    # which splits multi-sem waits into EventSemaphore chains -- TRN2 allows
    # at most 1 wait per instruction and the Tile tail drain accumulates one
    # wait per ticked semaphore.
    nc = bacc.Bacc()
    in_d = nc.declare_dram_parameter("inp", [NCTRL, INW], f16, isOutput=False)
    out_d = nc.declare_dram_parameter("out", [ROWS, OUTW], f16, isOutput=True)

    with tile.TileContext(nc) as tc, ExitStack() as ctx:
        sb = ctx.enter_context(tc.tile_pool(name="sb", bufs=1))
        ps = ctx.enter_context(tc.tile_pool(name="ps", bufs=PSBUFS, space="PSUM"))

        # Input on 32 partitions: [t3b | bt].  t3b[j, 128d+x] is the
        # host-side left product for this core's x rows; bt[j, y] = B[y, j].
        # One DMA: 32 long partition lines minimize modeled transfer time.
        inp = sb.tile([NCTRL, INW], f16, tag="inp")
        if INSPLIT is None:
            nc.sync.dma_start(inp[:], in_d[:])
        else:
            nc.sync.dma_start(inp[:, 0:INSPLIT], in_d[:, 0:INSPLIT])
            nc.sync.dma_start(inp[:, INSPLIT:INW], in_d[:, INSPLIT:INW])

        out_sb = sb.tile([ROWS, OUTW], f16, tag="osb")

        # out[x, 3y+d]: the fused copy reads PSUM [(d, y)] and writes the
        # interleaved [(y, d)] SBUF layout (strided dest is free), casting
        # f32 -> fp16 on the way.
        ov = out_sb[:].rearrange("p (y d) -> p d y", d=DIM)
        copy_eng = [nc.vector.tensor_copy, nc.scalar.copy]
        for h, (ya, yb) in enumerate(YCH):
            w = yb - ya
            mode = CENG[h]
            if DIM * w <= 512:
                # all three channel matmuls share one PSUM bank; copies can
                # fuse across channels
                p = ps.tile([ROWS, 512], f32, tag="p")
                pd = [p[:, d * w:(d + 1) * w] for d in range(DIM)]
                pv = p[:, 0:DIM * w].rearrange("p (d y) -> p d y", d=DIM)
            else:
                # one PSUM bank per channel; per-channel copies only
                pt = [ps.tile([ROWS, 512], f32, tag="p", name=f"p{h}_{d}")
                      for d in range(DIM)]
                pd = [t[:, 0:w] for t in pt]
                pv = None
            for d in range(DIM):
                nc.tensor.matmul(
                    pd[d], inp[:, d * ROWS:(d + 1) * ROWS],
                    inp[:, DIM * ROWS + ya:DIM * ROWS + yb])
            if mode in (0, 1) and pv is not None:
                copy_eng[mode](ov[:, :, ya:yb], pv)
            else:
                if mode in (0, 1):           # no fusing possible: alternate
                    splits, engs = [0, 1, 2, 3], [mode, 1 - mode, mode]
                else:
                    q, r = divmod(mode - 2, 2)
                    s = 2 - q                # channels on first engine: d < s
                    splits, engs = [0, s, 3], [r, 1 - r]
                for i in range(len(engs)):
                    a, b = splits[i], min(splits[i + 1], DIM)
                    if a >= b:
                        continue
                    if pv is not None:
                        copy_eng[engs[i]](ov[:, a:b, ya:yb], pv[:, a:b])
                    else:
                        for d in range(a, b):
                            copy_eng[engs[i]](ov[:, d, ya:yb], pd[d])
        for g, grp in enumerate(DMAG):
            ya, yb = YCH[grp[0]][0], YCH[grp[-1]][1]
            dma_eng = nc.sync if DMAENG[g] == 0 else nc.scalar
            dma_eng.dma_start(out_d[:, DIM * ya:DIM * yb],
                              out_sb[:, DIM * ya:DIM * yb])

    # Run Bacc's compile pipeline (wait legalization, register allocation)
    # before the BIR is serialized for the compiler.
    nc.finalize()
    _NC_CACHE["nc"] = nc
    return nc


# ------------------------------------------------------------------- wrapper
def _make_in_maps(control_points, knots_x, knots_y):
    cp = np.asarray(control_points, dtype=np.float32)
    A = _dense_basis_matrix(knots_x)                      # (1024, 32) [x, i]
    B = _dense_basis_matrix(knots_y)                      # (1024, 32) [y, j]
    # T3[j, d, x] = sum_i cp[i, j, d] * A[x, i] -- the tiny left product.
    T3 = np.einsum('ijd,xi->jdx', cp.astype(np.float64),
                   A.astype(np.float64))                  # (32, 3, 1024)
    bt = B.T.astype(np.float16)                           # (32, 1024)
    return [
        {
            "inp": np.concatenate(
                [T3[:, :, c * ROWS:(c + 1) * ROWS].reshape(NCTRL, DIM * ROWS)
                 .astype(np.float16), bt], axis=1),
        }
        for c in range(NCORES)
    ]


def kernel(control_points, knots_x, knots_y):
    from concourse.bass_utils import run_bass_kernel_spmd

    in_maps = _make_in_maps(control_points, knots_x, knots_y)
    nc = _build_nc()
    res = run_bass_kernel_spmd(nc, in_maps, core_ids=list(range(NCORES)))
    out = np.concatenate([res.results[c]["out"] for c in range(NCORES)], axis=0)
    return out.reshape(1, EOUT, EOUT, DIM).astype(np.float32)
